# revision 6
# baseline (speedup 1.0000x reference)
"""Multi-head attention (B=8, S=1024, D=1024, H=16, dk=dv=64) on 8 TRN2 cores.

Sharding: data-parallel over batch — core b computes batch element b end to
end; no collectives. Host-side prep transposes activations/weights into the
layouts TensorE needs (contraction dim on partitions); all matmuls run on
device.

v1 restructure vs baseline:
  - scores accumulate into [128,1024] 2-bank PSUM tiles; exp runs as one
    N=1024 ScalarE ACT per tile (halves the 352-cycle fixed ACT overhead)
  - per-iteration matmul interleave (q-proj | scores | pv-g0 per s2 step,
    then k-proj | pv-g1) keeps the PE FIFO fed while ScalarE drains exps
  - projection/fc psum groups are [128,1024]; single merged DVE copy each

v2 on top of v1 (from trace: per-iteration 2.3us PE gaps at the proj psum
handoff re-throttled HAM ~99us; 10us tail stall on the norm(7) chain):
  - step 7 of each phase runs the projection matmuls FIRST so the qT/kT
    evacuation copy becomes ready before the pv/ctx copies (the Tile
    scheduler orders same-engine work by dep readiness, not emission)
  - kT evacuation moved to ScalarE (idle through phase 2); qT stays on
    DVE but now wins the scheduling race
  - pv_evac copies the denominator rows before the ctx block so the
    reciprocal can issue early
  - the fc runs as pipelined filler through the attention tail: groups
    feed 2 matmuls per tail step while scores(7)/pv(6)/pv(7) drain, and
    ~30 deferred matmuls cover the final norm(7) chain (reciprocal on
    DVE) so the PE never idles
  - vpv ones-column memsets moved to GpSimd (DVE was backlogged early)
"""

import os

import numpy as np

import concourse.bacc as bacc
import concourse.mybir as mybir
import concourse.tile as tile
from concourse.bass_utils import run_bass_kernel_spmd

_LDW_OPT = os.environ.get("KMHA_LDW_OPT", "0") == "1"


def _install_ldw_opt():
    import concourse.bass_utils as bu

    if getattr(bu, "_kmha_ldw_patched", False):
        return
    orig = bu.run_command

    def patched(argv, **kw):
        argv = ["--enable-ldw-opt=true" if a == "--enable-ldw-opt=false" else a
                for a in argv]
        return orig(argv, **kw)

    bu.run_command = patched
    bu._kmha_ldw_patched = True

S = 1024
D = 1024
H = 16
DK = 64
P = 128
NT = S // P          # 8 seq/feature tiles
NCH = 2              # 512-wide free-dim chunks
CH = S // NCH        # 512
F32 = mybir.dt.float32
BF16 = mybir.dt.bfloat16
EXP = mybir.ActivationFunctionType.Exp

_CACHE = {}


def _build():
    nc = bacc.Bacc("TRN2", target_bir_lowering=False, debug=False)
    xqt = nc.dram_tensor("xqt", [D, S], BF16, kind="ExternalInput").ap()
    xkt = nc.dram_tensor("xkt", [D, S], BF16, kind="ExternalInput").ap()
    xvt = nc.dram_tensor("xvt", [D, S], BF16, kind="ExternalInput").ap()
    wqt = nc.dram_tensor("wqt", [D, D], BF16, kind="ExternalInput").ap()
    wkt = nc.dram_tensor("wkt", [D, D], BF16, kind="ExternalInput").ap()
    wvt = nc.dram_tensor("wvt", [D, D], BF16, kind="ExternalInput").ap()
    wft = nc.dram_tensor("wft", [D, D], BF16, kind="ExternalInput").ap()
    out = nc.dram_tensor("out", [S, D], F32, kind="ExternalOutput").ap()

    from contextlib import ExitStack

    with tile.TileContext(nc) as tc:
        with (
            tc.tile_pool(name="persist", bufs=1) as pp,
            tc.tile_pool(name="psum", bufs=2, space="PSUM") as psp,
        ):
            qT = [pp.tile([P, S], BF16, tag=f"qT{t}", name=f"qT{t}")
                  for t in range(NT)]
            kT = [pp.tile([P, S], BF16, tag=f"kT{t}", name=f"kT{t}")
                  for t in range(NT)]
            # v natural layout, ones column after each head (softmax denom)
            vpv = [pp.tile([P, H * (DK + 1)], BF16, tag=f"v{t}", name=f"v{t}")
                   for t in range(NT)]
            ctxT = [pp.tile([P, S], BF16, tag=f"c{t}", name=f"c{t}")
                    for t in range(NT)]
            ones1 = pp.tile([P, 1], BF16, tag="ones1", name="ones1")

            with ExitStack() as stk:
                ap_ = stk.enter_context(tc.tile_pool(name="attn", bufs=2))
                xtq = [ap_.tile([P, S], BF16, tag="xtq", name="xtq", bufs=8)
                       for _ in range(NT)]
                xtk = [ap_.tile([P, S], BF16, tag="xtk", name="xtk", bufs=8)
                       for _ in range(NT)]

                # ---- v projection (sc-tag 2-bank psum groups) ----
                with tc.tile_pool(name="vld", bufs=8) as vp:
                    xts = [vp.tile([P, S], BF16, tag="xt", name="xt")
                           for _ in range(NT)]
                    ws = [vp.tile([P, D], BF16, tag="w", name="w")
                          for _ in range(NT)]
                    for t in range(NT):
                        nc.sync.dma_start(out=xts[t][:],
                                          in_=xvt[t * P:(t + 1) * P, :])
                        nc.sync.dma_start(out=ws[t][:],
                                          in_=wvt[t * P:(t + 1) * P, :])
                    for t in range(NT):
                        nc.sync.dma_start(out=xtq[t][:],
                                          in_=xqt[t * P:(t + 1) * P, :])
                        nc.sync.dma_start(out=xtk[t][:],
                                          in_=xkt[t * P:(t + 1) * P, :])
                    # ones columns for the softmax denominators: memset the
                    # whole tile (contiguous, safe) — the vproj copies then
                    # overwrite the 64-wide v blocks, leaving column 64 of
                    # each head block at 1.0
                    for t in range(NT):
                        nc.vector.memset(vpv[t][:], 1.0)
                    nc.vector.memset(ones1[:], 1.0)
                    for s2 in range(NT):
                        ps = psp.tile([P, S], F32, tag="sc", name="vps")
                        for d in range(NT):
                            for c in range(NCH):
                                nc.tensor.matmul(
                                    ps[:, c * CH:(c + 1) * CH],
                                    lhsT=xts[d][:, s2 * P:(s2 + 1) * P],
                                    rhs=ws[d][:, c * CH:(c + 1) * CH],
                                    start=(d == 0),
                                    stop=(d == NT - 1),
                                )
                        nc.vector.tensor_copy(
                            vpv[s2][:, 0:H * 65].rearrange(
                                "p (h x) -> p h x", x=65)[:, :, 0:64],
                            ps[:].rearrange("p (h x) -> p h x", x=64),
                        )

                # ---- software-pipelined a-loop ----
                # iteration a: qk-proj(a) | scores(a-1) | pv(a-2) | norm(a-3)
                exps_hist = {}   # (gen a) -> {(s2, c): exp tile [P, 1024]}
                rrec_hist = {}

                def scores_step(a, s2, exps):
                    # one s2-row of scores for head pair a. Two 2-bank psum
                    # tiles: tile g holds [c0 | c1] for head g. The (g0,c)/
                    # (g1,c) matmul pairs run concurrently via PE row
                    # tiling; exp is one N=1024 ACT per head.
                    # ex_g0 tags are single-buffered: their last reader
                    # (pv g0 of gen a-2) is emitted earlier in this same
                    # step, so the WAR dep is tight but never cross-phase.
                    scs = [psp.tile([P, S], F32, tag="sc", name="sc")
                           for _ in range(2)]
                    for c in range(NCH):
                        for g in range(2):
                            nc.tensor.matmul(
                                scs[g][:, c * CH:(c + 1) * CH],
                                lhsT=kT[a][g * DK:(g + 1) * DK,
                                           s2 * P:(s2 + 1) * P],
                                rhs=qT[a][g * DK:(g + 1) * DK,
                                          c * CH:(c + 1) * CH],
                                start=True, stop=True,
                                tile_position=(g * DK, 0),
                            )
                    for g in range(2):
                        ex = ap_.tile([P, S], BF16, tag=f"ex{g}_{s2}",
                                      name=f"ex{g}_{s2}",
                                      bufs=(1 if g == 0 else 2))
                        nc.scalar.activation(ex[:], scs[g][:], EXP,
                                             scale=0.125)
                        exps[(s2, g)] = ex

                def pv_mms(a, g, s2, c, pvt, exps):
                    h = 2 * a + g
                    nc.tensor.matmul(
                        pvt[0:DK + 1, :],
                        lhsT=vpv[s2][:, h * 65:(h + 1) * 65],
                        rhs=exps[(s2, g)][:, c * CH:(c + 1) * CH],
                        start=(s2 == 0),
                        stop=(s2 == NT - 1),
                    )

                def pv_rows(g, pvs, rows):
                    # denominator rows split out: the reciprocal chain is
                    # the long pole, ctx copies have slack
                    for c in range(NCH):
                        ri = 32 * (2 * g + c)
                        nc.vector.tensor_copy(
                            rows[ri:ri + 1, :],
                            pvs[c][DK:DK + 1, :])

                def pv_ctx(a, g, pvs):
                    for c in range(NCH):
                        nc.vector.tensor_copy(
                            ctxT[a][g * DK:(g + 1) * DK,
                                    c * CH:(c + 1) * CH],
                            pvs[c][0:DK, :])

                def pv_evac(a, g, pvs, rows):
                    pv_rows(g, pvs, rows)
                    pv_ctx(a, g, pvs)

                def norm_phase(a, rrec, gs=(0, 1)):
                    # both chunk reciprocal rows staged into one [1,1024]
                    # tile at partition 0, one GpSimd broadcast per head,
                    # then scale ctx in place on DVE
                    for g in gs:
                        r0 = ap_.tile([1, S], F32, tag="r0", name="r0",
                                      bufs=2)
                        for c in range(NCH):
                            ri = 32 * (2 * g + c)
                            nc.vector.tensor_copy(
                                r0[:, c * CH:(c + 1) * CH],
                                rrec[ri:ri + 1, :])
                        rb = ap_.tile([P, S], F32, tag="rb", name="rb",
                                      bufs=2)
                        nc.gpsimd.partition_broadcast(rb[:], r0[:])
                        for c in range(NCH):
                            sl = ctxT[a][g * DK:(g + 1) * DK,
                                         c * CH:(c + 1) * CH]
                            nc.vector.tensor_mul(
                                sl, sl,
                                rb[g * DK:(g + 1) * DK,
                                   c * CH:(c + 1) * CH])

                for a in range(NT):
                    if a >= 1:
                        exps_hist[a - 1] = {}
                    if a >= 2:
                        pvs_g0 = [psp.tile([P, CH], F32, tag="pv", name="pv")
                                  for _ in range(NCH)]

                    # ---- phase 1: q-proj | scores(a-1) | pv(a-2) g0 ----
                    qp = psp.tile([P, S], F32, tag="proj", name="qp", bufs=1)
                    for t in range(NT):
                        wt = ap_.tile([P, P], BF16, tag="wqk", name="wqk",
                                      bufs=18)
                        nc.sync.dma_start(
                            out=wt[:],
                            in_=wqt[t * P:(t + 1) * P, a * P:(a + 1) * P])

                        def q_mms():
                            for c in range(NCH):
                                nc.tensor.matmul(
                                    qp[:, c * CH:(c + 1) * CH],
                                    lhsT=wt[:],
                                    rhs=xtq[t][:, c * CH:(c + 1) * CH],
                                    start=(t == 0),
                                    stop=(t == NT - 1),
                                )

                        # last step: projection first so the qT evacuation
                        # copy's dependency fires before the pv/ctx copies
                        # (the scheduler orders DVE work by readiness)
                        if t == NT - 1:
                            q_mms()
                        # pv g0 before scores: frees the single-buffered
                        # ex_g0 tile this step's ACT will rewrite
                        if a >= 2:
                            for c in range(NCH):
                                pv_mms(a - 2, 0, t, c, pvs_g0[c],
                                       exps_hist[a - 2])
                        if a >= 1:
                            scores_step(a - 1, t, exps_hist[a - 1])
                        if t < NT - 1:
                            q_mms()
                    nc.vector.tensor_copy(qT[a][:], qp[:])
                    if a >= 2:
                        rows = ap_.tile([97, CH], F32, tag="rows",
                                        name="rows")
                        pv_evac(a - 2, 0, pvs_g0, rows)
                        pvs_g1 = [psp.tile([P, CH], F32, tag="pv", name="pv")
                                  for _ in range(NCH)]
                    # norm(a-3) emitted after the phase-1 evacuations: its
                    # DVE ops sort behind the psum-releasing copies and
                    # run during phase 2 instead
                    if a >= 3:
                        norm_phase(a - 3, rrec_hist.pop(a - 3))
                    # ---- phase 2: k-proj | pv(a-2) g1 ----
                    kp = psp.tile([P, S], F32, tag="proj", name="kp", bufs=1)
                    for t in range(NT):
                        wt = ap_.tile([P, P], BF16, tag="wqk", name="wqk",
                                      bufs=18)
                        nc.sync.dma_start(
                            out=wt[:],
                            in_=wkt[t * P:(t + 1) * P, a * P:(a + 1) * P])

                        def k_mms():
                            for c in range(NCH):
                                nc.tensor.matmul(
                                    kp[:, c * CH:(c + 1) * CH],
                                    lhsT=wt[:],
                                    rhs=xtk[t][:, c * CH:(c + 1) * CH],
                                    start=(t == 0),
                                    stop=(t == NT - 1),
                                )

                        if t == NT - 1:
                            k_mms()
                        if a >= 2:
                            for c in range(NCH):
                                pv_mms(a - 2, 1, t, c, pvs_g1[c],
                                       exps_hist[a - 2])
                        if t < NT - 1:
                            k_mms()
                    # kT evacuation on ScalarE: it is idle through phase 2,
                    # so the copy starts the moment the k d=7 matmul lands
                    nc.scalar.copy(kT[a][:], kp[:])
                    if a >= 2:
                        # ctx copies before the reciprocal: they release
                        # the pv psum bufs the next iteration is waiting on
                        pv_evac(a - 2, 1, pvs_g1, rows)
                        nc.vector.reciprocal(rows[:], rows[:])
                        rrec_hist[a - 2] = rows
                        exps_hist.pop(a - 2)

                # fc weights reuse the q input slots freed by the final
                # projections — loads overlap the attention tail
                wf = [ap_.tile([P, S], BF16, tag="xtq", name="wf", bufs=8)
                      for _ in range(NT)]
                for t in range(NT):
                    nc.sync.dma_start(out=wf[t][:],
                                      in_=wft[t * P:(t + 1) * P, :])

                # ---- attention tail with fc pipelined as PE filler ----
                # fc psum groups: s1=0 on the freed proj slot, s1=1,2 on
                # the sc slots as scores(7) drains, s1=3 on the pv slots
                # after pv(7) evacuates; s1=4..7 rotate afterwards.
                fc_ps = {}

                def fc_mm(s1, ct, start, stop):
                    fp = fc_ps[s1]
                    for c in range(NCH):
                        dst = (fp[c][:] if isinstance(fp, list)
                               else fp[:, c * CH:(c + 1) * CH])
                        nc.tensor.matmul(
                            dst,
                            lhsT=ctxT[ct][:, s1 * P:(s1 + 1) * P],
                            rhs=wf[ct][:, c * CH:(c + 1) * CH],
                            start=start,
                            stop=stop,
                        )

                def fc_evac(s1):
                    fp = fc_ps[s1]
                    for c in range(NCH):
                        src = (fp[c][:] if isinstance(fp, list)
                               else fp[:, c * CH:(c + 1) * CH])
                        ob = ap_.tile([P, CH], F32, tag="xtk", name="ob",
                                      bufs=8)
                        nc.vector.tensor_copy(ob[:], src)
                        nc.sync.dma_start(
                            out=out[s1 * P:(s1 + 1) * P, c * CH:(c + 1) * CH],
                            in_=ob[:],
                        )

                exps_hist[NT - 1] = {}
                # T6 phase 1: pv(6) g0 | scores(7) | fc0 ct=t filler
                pvs_g0 = [psp.tile([P, CH], F32, tag="pv", name="pv")
                          for _ in range(NCH)]
                norm_phase(NT - 3, rrec_hist.pop(NT - 3))
                fc_ps[0] = psp.tile([P, S], F32, tag="proj", name="fc0",
                                    bufs=1)
                for t in range(NT):
                    for c in range(NCH):
                        pv_mms(NT - 2, 0, t, c, pvs_g0[c],
                               exps_hist[NT - 2])
                    scores_step(NT - 1, t, exps_hist[NT - 1])
                    if t < 6:
                        fc_mm(0, t, start=(t == 0), stop=False)
                rows6 = ap_.tile([97, CH], F32, tag="rows", name="rows")
                pv_evac(NT - 2, 0, pvs_g0, rows6)
                # T6 phase 2: pv(6) g1 | fc1 ct=t filler
                pvs_g1 = [psp.tile([P, CH], F32, tag="pv", name="pv")
                          for _ in range(NCH)]
                fc_ps[1] = psp.tile([P, S], F32, tag="sc", name="fc1")
                for t in range(NT):
                    for c in range(NCH):
                        pv_mms(NT - 2, 1, t, c, pvs_g1[c],
                               exps_hist[NT - 2])
                    if t < 6:
                        fc_mm(1, t, start=(t == 0), stop=False)
                pv_evac(NT - 2, 1, pvs_g1, rows6)
                nc.vector.reciprocal(rows6[:], rows6[:])
                exps_hist.pop(NT - 2)
                # T7 phase 1: pv(7) g0 | fc2 filler | g1-denominator rows.
                # The g1 softmax denominators are accumulated early via
                # M=1 ones-matmuls into the free partition-96 rows of the
                # pv psum bank, so the a=7 reciprocal (the 3.4us DVE
                # divide) runs hidden under phase 2 instead of gating fc.
                norm_phase(NT - 2, rows6)
                pvs_g0 = [psp.tile([P, CH], F32, tag="pv", name="pv")
                          for _ in range(NCH)]
                fc_ps[2] = psp.tile([P, S], F32, tag="sc", name="fc2")
                for t in range(NT):
                    for c in range(NCH):
                        pv_mms(NT - 1, 0, t, c, pvs_g0[c],
                               exps_hist[NT - 1])
                    for c in range(NCH):
                        nc.tensor.matmul(
                            pvs_g0[c][96:97, :],
                            lhsT=ones1[:],
                            rhs=exps_hist[NT - 1][(t, 1)][:,
                                                          c * CH:(c + 1) * CH],
                            start=False, stop=False,
                            tile_position=(0, 96),
                            skip_group_check=True,
                        )
                    if t < 6:
                        fc_mm(2, t, start=(t == 0), stop=False)
                rows7 = ap_.tile([97, CH], F32, tag="rows", name="rows")
                pv_rows(0, pvs_g0, rows7)
                for c in range(NCH):
                    nc.vector.tensor_copy(rows7[32 * (2 + c):32 * (2 + c) + 1, :],
                                          pvs_g0[c][96:97, :])
                pv_ctx(NT - 1, 0, pvs_g0)
                nc.vector.reciprocal(rows7[:], rows7[:])
                # T7 phase 2: pv(7) g1 (dense, exps all ready); the g0
                # norm muls run underneath (ctxT[7] g0-half + recip ready)
                norm_phase(NT - 1, rows7, gs=(0,))
                pvs_g1 = [psp.tile([P, CH], F32, tag="pv", name="pv")
                          for _ in range(NCH)]
                for t in range(NT):
                    for c in range(NCH):
                        pv_mms(NT - 1, 1, t, c, pvs_g1[c],
                               exps_hist[NT - 1])
                pv_ctx(NT - 1, 1, pvs_g1)
                exps_hist.pop(NT - 1)
                # remaining norm(7) work: just the g1 broadcast+mul; the
                # PE covers it with deferred ct=6 contributions and fc3
                norm_phase(NT - 1, rows7, gs=(1,))
                for s1 in (0, 1, 2):
                    fc_mm(s1, 6, start=False, stop=False)
                fc_ps[3] = [psp.tile([P, CH], F32, tag="pv", name="fc3")
                            for _ in range(NCH)]
                for ct in range(NT - 1):
                    fc_mm(3, ct, start=(ct == 0), stop=False)
                for s1 in (0, 1, 2, 3):
                    fc_mm(s1, NT - 1, start=False, stop=True)
                    fc_evac(s1)
                for s1, tag in ((4, "sc"), (5, "proj"), (6, "sc"), (7, "sc")):
                    fc_ps[s1] = psp.tile([P, S], F32, tag=tag, name=f"fc{s1}",
                                         bufs=(1 if tag == "proj" else 2))
                    for ct in range(NT):
                        fc_mm(s1, ct, start=(ct == 0), stop=(ct == NT - 1))
                    fc_evac(s1)

    nc.compile()
    return nc


def run(inputs, trace=False):
    """inputs: dict with Q,K,V [8,1024,1024] and WQ,WK,WV,Wfc [1024,1024].
    Returns (out [8,1024,1024] fp32, exec_time_ns or None)."""
    if _LDW_OPT:
        _install_ldw_opt()
    if "nc" not in _CACHE:
        _CACHE["nc"] = _build()
    nc = _CACHE["nc"]

    import ml_dtypes
    bf16 = ml_dtypes.bfloat16
    f32 = np.float32
    wqt = np.ascontiguousarray(np.asarray(inputs["WQ"], dtype=f32).T.astype(bf16))
    wkt = np.ascontiguousarray(np.asarray(inputs["WK"], dtype=f32).T.astype(bf16))
    wvt = np.ascontiguousarray(np.asarray(inputs["WV"], dtype=f32).T.astype(bf16))
    wft = np.ascontiguousarray(np.asarray(inputs["Wfc"], dtype=f32).T.astype(bf16))
    Q = np.asarray(inputs["Q"], dtype=f32)
    K = np.asarray(inputs["K"], dtype=f32)
    V = np.asarray(inputs["V"], dtype=f32)

    in_maps = [
        {
            "xqt": np.ascontiguousarray(Q[b].T.astype(bf16)),
            "xkt": np.ascontiguousarray(K[b].T.astype(bf16)),
            "xvt": np.ascontiguousarray(V[b].T.astype(bf16)),
            "wqt": wqt, "wkt": wkt, "wvt": wvt, "wft": wft,
        }
        for b in range(8)
    ]
    res = run_bass_kernel_spmd(nc, in_maps, core_ids=list(range(8)), trace=trace)
    out = np.stack([res.results[b]["out"] for b in range(8)], axis=0)
    return out.astype(np.float32), res.exec_time_ns


def kernel(**inputs):
    return run(inputs, trace=False)[0]
